# revision 40
# baseline (speedup 1.0000x reference)
"""Relative-position attention (TransformerXL-style) on 8 TRN2 NeuronCores.

Sharding: data-parallel over batch (b=8 -> 1 batch element per core); weights
replicated. No collectives needed.

Per-core pipeline (n=1024, dim=512, heads=8, d_head=64):
  qT = Wq^T x^T, kT = Wk^T x^T   [inner, n]   (bf16 matmuls, fp32 psum)
  v  = x Wv                      [n, inner]
  per (head h, 128-row query tile m):
    S_psum[128, 1024] = qTh_m^T kTh           (2 matmuls into one 2-bank tile)
    T_psum[128, w]    = qTh_m^T relT[:, band]   (2 matmuls)
    t8 (fp8) <- T_psum + clip-tail fills ; pos8 via the diagonal
        SBUF->SBUF skew DMA (fp8: half the bytes of bf16)
    S_psum += pos8  (identity matmul, fp8 moving operand)
    P_sb (bf16), z = exp(0.125 * S_psum)  (single ACT op, fused row-sum;
                                           no DVE z-add needed)
    diag = identity * (1/z)  (per-partition scalar mul -> diag(r))
    PT_jb = P[:, jb]^T @ diag(r)   (PE transpose; folds softmax normalization)
    av[128, 64] += PT_jb-stationary @ v_jb_h   (8 matmuls, N=64 moving: half
                                                the PE cost of v-stationary)
  o_att[m] -> toT via DMA-xbar transpose (pipelined one m deep)
  out_m[128, 512] = toT-blocks^T Wo + ones^T bo  (5 matmuls, K=1 bias trick)

The rel-pos table is host-preprocessed into relT[d, c] = rel_emb[1024 -
clip(c - 511, 0, 1024), d] so that pos_attn[i, j] = (q_i . relT[:, j - i +
1023]) and clipping is baked into the padded table.
"""
import sys

sys.path.insert(0, "/opt/trn_rl_repo")

import numpy as np

import concourse.bass as bass
import concourse.bacc as bacc
import concourse.mybir as mybir
import concourse.tile as tile
from concourse.ap import AP
from concourse.bass_utils import run_bass_kernel_spmd

F32 = mybir.dt.float32
BF16 = mybir.dt.bfloat16
FP8 = mybir.dt.float8e4
DR = mybir.MatmulPerfMode.DoubleRow

B, N, DIM = 8, 1024, 512
HEADS, DH = 8, 64
INNER = HEADS * DH
MAX_POS = 512
RELW = 2 * MAX_POS + 1        # 1025 rel-emb rows
RELTW = 2047                  # extended/clip-padded table width
TW = 1151                     # per-query-tile T width (1024 + 127)
TWPAD = 1152
KC = DIM // 128               # 4 contraction chunks
MT = N // 128                 # 8 query row tiles
SCALE = DH ** -0.5

_CACHE = {}

import os
CFG = {
    "asb": int(os.environ.get("K_ASB", "5")),
    "s": int(os.environ.get("K_S", "1")),
    "t": int(os.environ.get("K_T", "2")),
    "pt": int(os.environ.get("K_PT", "2")),
    "ot": int(os.environ.get("K_OT", "2")),
    "ptsb": int(os.environ.get("K_PTSB", "4")),
    "drt": int(os.environ.get("K_DRT", "1")),   # DoubleRow T matmul
    "dri": int(os.environ.get("K_DRI", "1")),   # DoubleRow ident matmul
}


def _build_nc():
    nc = bacc.Bacc()
    xT_in = nc.declare_dram_parameter("xT", [DIM, N], BF16, isOutput=False)
    wq_in = nc.declare_dram_parameter("wq", [DIM, INNER], BF16, isOutput=False)
    wk_in = nc.declare_dram_parameter("wk", [DIM, INNER], BF16, isOutput=False)
    wv_in = nc.declare_dram_parameter("wv", [DIM, INNER], BF16, isOutput=False)
    wo_in = nc.declare_dram_parameter("wo", [INNER, DIM], BF16, isOutput=False)
    rel2_in = nc.declare_dram_parameter("relT2", [64, 2 * RELTW], FP8, isOutput=False)
    rel8_in = nc.declare_dram_parameter("relT8", [128, RELTW], FP8, isOutput=False)
    id8_in = nc.declare_dram_parameter("id8", [128, 128], FP8, isOutput=False)
    bo_in = nc.declare_dram_parameter("bo", [1, DIM], BF16, isOutput=False)
    ident_in = nc.declare_dram_parameter("ident", [128, 128], BF16, isOutput=False)
    id2_in = nc.declare_dram_parameter("id2", [64, 256], FP8, isOutput=False)
    out_ext = nc.declare_dram_parameter("out", [N, DIM], F32, isOutput=True)

    with tile.TileContext(nc) as tc:
        with tc.tile_pool(name="persist", bufs=1) as pp:
            # ---- load persistent operands ----
            xT_sb = [pp.tile([128, N], BF16, name=f"xT{k}") for k in range(KC)]
            wq_sb = [pp.tile([128, INNER], BF16, name=f"wq{k}") for k in range(KC)]
            wk_sb = [pp.tile([128, INNER], BF16, name=f"wk{k}") for k in range(KC)]
            wv_sb = [pp.tile([128, INNER], BF16, name=f"wv{k}") for k in range(KC)]
            wo_sb = [pp.tile([128, DIM], BF16, name=f"wo{k}") for k in range(KC)]
            rel2_sb = pp.tile([64, 2 * RELTW], FP8)
            rel8_sb = pp.tile([128, RELTW], FP8)
            id8_sb = pp.tile([128, 128], FP8)
            bo_sb = pp.tile([1, DIM], BF16)
            ident_sb = pp.tile([128, 128], BF16)
            id2_sb = pp.tile([64, 256], FP8)
            ones_sb = pp.tile([1, 128], BF16)
            onesw_sb = pp.tile([128, 512], BF16)
            for k in range(KC):
                nc.sync.dma_start(out=xT_sb[k][:], in_=xT_in[128 * k:128 * (k + 1), :])
                nc.sync.dma_start(out=wq_sb[k][:], in_=wq_in[128 * k:128 * (k + 1), :])
                nc.sync.dma_start(out=wk_sb[k][:], in_=wk_in[128 * k:128 * (k + 1), :])
            for k in range(KC):
                nc.sync.dma_start(out=wv_sb[k][:], in_=wv_in[128 * k:128 * (k + 1), :])
            nc.sync.dma_start(out=rel2_sb[:], in_=rel2_in[:])
            nc.sync.dma_start(out=rel8_sb[:], in_=rel8_in[:])
            nc.sync.dma_start(out=id8_sb[:], in_=id8_in[:])
            nc.sync.dma_start(out=ident_sb[:], in_=ident_in[:])
            nc.sync.dma_start(out=id2_sb[:], in_=id2_in[:])
            for k in range(KC):
                nc.sync.dma_start(out=wo_sb[k][:], in_=wo_in[128 * k:128 * (k + 1), :])
            nc.sync.dma_start(out=bo_sb[:], in_=bo_in[:])
            nc.gpsimd.memset(ones_sb[:], 1.0)
            nc.gpsimd.memset(onesw_sb[:], 1.0)

            # ---- projections ----
            qT_sb = [pp.tile([128, N], BF16, name=f"qT{t}") for t in range(KC)]
            kT_sb = [pp.tile([128, N], BF16, name=f"kT{t}") for t in range(KC)]
            q8_sb = [pp.tile([128, N], FP8, name=f"q8{t}") for t in range(KC)]
            q2_sb = [pp.tile([64, 2 * N], FP8, name=f"q2{t}") for t in range(KC)]
            v_sb = [pp.tile([128, INNER], BF16, name=f"v{t}") for t in range(MT)]
            o_att = [pp.tile([128, INNER], BF16, name=f"oatt{t}") for t in range(MT)]

            with tc.tile_pool(name="proj_ps", bufs=4, space="PSUM") as proj_ps:
                for t in range(KC):          # qT / kT tiles: inner rows 128t..
                    for jc in range(2):      # n column chunks of 512
                        for which, w_sb, dst in (("q", wq_sb, qT_sb), ("k", wk_sb, kT_sb)):
                            ps = proj_ps.tile([128, 512], F32, tag="pps",
                                              name=f"ps{which}{t}{jc}")
                            for k in range(KC):
                                nc.tensor.matmul(
                                    ps[:],
                                    w_sb[k][:, 128 * t:128 * (t + 1)],
                                    xT_sb[k][:, 512 * jc:512 * (jc + 1)],
                                    start=(k == 0), stop=(k == KC - 1))
                            nc.vector.tensor_copy(dst[t][:, 512 * jc:512 * (jc + 1)], ps[:])
                            if which == "q":
                                nc.scalar.copy(q8_sb[t][:, 512 * jc:512 * (jc + 1)], ps[:])
                for t in range(MT):          # v tiles: n rows 128t..
                    ps = proj_ps.tile([128, 512], F32, tag="pps", name=f"psv{t}")
                    for k in range(KC):
                        nc.tensor.matmul(
                            ps[:],
                            xT_sb[k][:, 128 * t:128 * (t + 1)],
                            wv_sb[k][:],
                            start=(k == 0), stop=(k == KC - 1))
                    if t % 2 == 0:
                        nc.scalar.copy(v_sb[t][:], ps[:])
                    else:
                        nc.vector.tensor_copy(v_sb[t][:], ps[:])
                # pair-pack q for DoubleRow: q2[k2, e*N + i] = q8[2*k2 + e, i]
                # (flat-identical layouts; the DMA copy relabels partitions).
                # memset first: the partition-crossing flat write confuses the
                # interp's init tracking (values are correct).
                for t in range(KC):
                    nc.gpsimd.memset(q2_sb[t][:], 0.0)
                for t in range(KC):
                    nc.sync.dma_start(
                        out=AP(q2_sb[t].tensor, q2_sb[t].offset, [[N, 128], [1, N]]),
                        in_=AP(q8_sb[t].tensor, q8_sb[t].offset, [[N, 128], [1, N]]))

            # ---- attention ----
            with tc.tile_pool(name="attn_sb", bufs=CFG["asb"]) as asb, \
                 tc.tile_pool(name="attn_ps", bufs=CFG["s"], space="PSUM") as aps, \
                 tc.tile_pool(name="tp_ps", bufs=CFG["t"], space="PSUM") as tps, \
                 tc.tile_pool(name="pt_ps", bufs=CFG["pt"], space="PSUM") as ptps, \
                 tc.tile_pool(name="ot_ps", bufs=CFG["ot"], space="PSUM") as otps, \
                 tc.tile_pool(name="fin_sb", bufs=2) as osb:
                to_pend = []

                def oproj(m, toT):
                    # (shares the pt_ps rotation buffers; same tile shape)
                    o_ps = ptps.tile([128, DIM], F32, tag="pt_ps", name="o_ps")
                    for g in range(KC):
                        nc.tensor.matmul(
                            o_ps[:],
                            toT[:, 128 * g:128 * (g + 1)],
                            wo_sb[g][:],
                            start=(g == 0), stop=False)
                    nc.tensor.matmul(o_ps[:], ones_sb[:], bo_sb[:],
                                     start=False, stop=True)
                    o_sb = osb.tile([128, DIM], F32, name="o_sb")
                    if m % 2 == 0:
                        nc.scalar.copy(o_sb[:], o_ps[:])
                    else:
                        nc.vector.tensor_copy(o_sb[:], o_ps[:])
                    nc.sync.dma_start(
                        out=out_ext[128 * m:128 * (m + 1), :], in_=o_sb[:])

                for m in range(MT):
                    for h in range(HEADS):
                        th, ph = h // 2, (h % 2) * 64
                        qh = qT_sb[th][ph:ph + 64, 128 * m:128 * (m + 1)]
                        s_ps = aps.tile([128, 1024], F32, name="s_ps")
                        for jc in range(2):
                            nc.tensor.matmul(
                                s_ps[:, 512 * jc:512 * (jc + 1)],
                                qh,
                                kT_sb[th][ph:ph + 64, 512 * jc:512 * (jc + 1)],
                                start=True, stop=False)
                        # rel-pos T tile via fp8 DoubleRow (0.5 cycles/row):
                        # only the unclipped band of relT2; clipped tails are
                        # constant per row (edge cols).
                        off = 896 - 128 * m
                        lo = max(0, 128 * m - 385)
                        hi = min(1150, 128 * m + 639)
                        w = hi - lo + 1
                        pb = 32 * (h % 2)   # partition base for q2/relT2 pairs
                        q2l = AP(q2_sb[th].tensor,
                                 q2_sb[th].offset + pb * 2 * N + 128 * m,
                                 [[2 * N, 32], [N, 2], [1, 128]])
                        t_sb = asb.tile([128, TWPAD], FP8, name="t_sb")
                        t_chunks = []
                        for ci, (c0, cw) in enumerate(((lo, 512), (lo + 512, w - 512))):
                            t_ps = tps.tile([128, 512], F32, tag="t_ps",
                                            name=f"t_ps{ci}")
                            t_chunks.append(t_ps)
                            if CFG["drt"]:
                                rel2l = AP(rel2_sb.tensor,
                                           rel2_sb.offset + pb * 2 * RELTW + off + c0,
                                           [[2 * RELTW, 32], [RELTW, 2], [1, cw]])
                                nc.tensor.matmul(
                                    t_ps[:, 0:cw], q2l, rel2l,
                                    start=True, stop=True, perf_mode=DR)
                            else:
                                nc.tensor.matmul(
                                    t_ps[:, 0:cw],
                                    q8_sb[th][ph:ph + 64, 128 * m:128 * (m + 1)],
                                    rel8_sb[ph:ph + 64, off + c0:off + c0 + cw],
                                    start=True, stop=True)
                            nc.vector.tensor_copy(t_sb[:, c0:c0 + cw], t_ps[:, 0:cw])
                        if lo > 0:    # low clip tail: rows of rel_emb[1024]
                            nc.vector.tensor_scalar_mul(
                                t_sb[:, 0:lo], onesw_sb[:, 0:lo],
                                t_chunks[0][:, 0:1])
                        if hi < 1150:  # high clip tail: rows of rel_emb[0]
                            nc.vector.tensor_scalar_mul(
                                t_sb[:, hi + 1:1151], onesw_sb[:, 0:1150 - hi],
                                t_chunks[1][:, w - 513:w - 512])
                        # Toeplitz skew straight into the pair-packed pos2
                        # layout (flat-identical): pos2 flat[p*1024 + j] =
                        # t_sb[p, j + 127 - p]
                        skew = AP(t_sb.tensor, t_sb.offset + 127,
                                  [[TWPAD - 1, 128], [1, N]])
                        if CFG["dri"]:
                            pos2 = asb.tile([64, 2 * N], FP8, name="pos2")
                            nc.gpsimd.memset(pos2[:], 0.0)
                            dst = AP(pos2.tensor, pos2.offset, [[N, 128], [1, N]])
                            nc.sync.dma_start(out=dst, in_=skew)
                            # S += pos, fp8 DoubleRow identity matmul
                            id2l = AP(id2_sb.tensor, id2_sb.offset,
                                      [[256, 64], [128, 2], [1, 128]])
                            for jc in range(2):
                                pos2l = AP(pos2.tensor, pos2.offset + 512 * jc,
                                           [[2 * N, 64], [N, 2], [1, 512]])
                                nc.tensor.matmul(
                                    s_ps[:, 512 * jc:512 * (jc + 1)],
                                    id2l, pos2l,
                                    start=False, stop=True, perf_mode=DR)
                        else:
                            pos8 = asb.tile([128, N], FP8, name="pos2")
                            nc.sync.dma_start(out=pos8[:], in_=skew)
                            for jc in range(2):
                                nc.tensor.matmul(
                                    s_ps[:, 512 * jc:512 * (jc + 1)],
                                    id8_sb[:],
                                    pos8[:, 512 * jc:512 * (jc + 1)],
                                    start=False, stop=True)
                        # softmax (no max-subtraction: logits are O(5));
                        # single ACT op with fused row-sum
                        p_sb = asb.tile([128, N], BF16, name="p_sb")
                        z_sb = asb.tile([128, 1], F32, name="z_sb")
                        nc.scalar.activation(
                            p_sb[:], s_ps[:],
                            mybir.ActivationFunctionType.Exp,
                            scale=SCALE, accum_out=z_sb[:])
                        r_sb = asb.tile([128, 1], F32, name="r_sb")
                        nc.vector.reciprocal(r_sb[:], z_sb[:])
                        diag_sb = asb.tile([128, 128], BF16, name="diag_sb")
                        nc.vector.tensor_scalar_mul(diag_sb[:], ident_sb[:], r_sb[:])
                        # P^T @ diag(r): 4 transposed blocks per PSUM bank
                        # tile, then AV with pt as the STATIONARY operand and
                        # v moving: N=64 per matmul (half the PE cost of the
                        # v-stationary orientation); out is [i, d].
                        av_ps = otps.tile([128, DH], F32, name="av_ps")
                        pt_sbs = []
                        for half in range(2):
                            pt_ps = ptps.tile([128, 512], F32, name="pt_ps")
                            for q in range(4):
                                jb = 4 * half + q
                                nc.tensor.matmul(
                                    pt_ps[:, 128 * q:128 * (q + 1)],
                                    p_sb[:, 128 * jb:128 * (jb + 1)],
                                    diag_sb[:], start=True, stop=True)
                            pt_sb = asb.tile([128, 512], BF16, name="pt_sb",
                                             bufs=CFG["ptsb"])
                            if half == 0:
                                nc.scalar.copy(pt_sb[:], pt_ps[:])
                            else:
                                nc.vector.tensor_copy(pt_sb[:], pt_ps[:])
                            pt_sbs.append(pt_sb)
                        for half in range(2):
                            for q in range(4):
                                jb = 4 * half + q
                                nc.tensor.matmul(
                                    av_ps[:],
                                    pt_sbs[half][:, 128 * q:128 * (q + 1)],
                                    v_sb[jb][:, DH * h:DH * (h + 1)],
                                    start=(jb == 0), stop=(jb == MT - 1))
                        if h % 2 == 0:
                            nc.scalar.copy(
                                o_att[m][:, DH * h:DH * (h + 1)], av_ps[:])
                        else:
                            nc.vector.tensor_copy(
                                o_att[m][:, DH * h:DH * (h + 1)], av_ps[:])

                    # ---- output projection, software-pipelined one m deep:
                    # launch the o_att[m] xbar transpose now, run the matmuls
                    # for m-1 (whose toT landed during this m's head loop).
                    toT = asb.tile([128, INNER], BF16, name="toT", bufs=3)
                    to_out = AP(toT.tensor, toT.offset,
                                [[INNER, 128], [128, KC], [1, 128]])
                    nc.sync.dma_start_transpose(to_out, o_att[m][:])
                    to_pend.append((m, toT))
                    if m > 0:
                        oproj(*to_pend.pop(0))

                for mm, tt in to_pend:
                    oproj(mm, tt)
    nc.compile()
    return nc


def _prep_inputs(x, Wq, Wkv, rel_emb, Wo, bo):
    import ml_dtypes
    tobf = lambda a: np.asarray(a, dtype=np.float32).astype(ml_dtypes.bfloat16)
    tof8 = lambda a: np.asarray(a, dtype=np.float32).astype(ml_dtypes.float8_e4m3)
    Wk = Wkv[:, :INNER]
    Wv = Wkv[:, INNER:]
    # relT[d, c] = rel_emb[1024 - clip(c - 511, 0, 1024), d]; pair-packed for
    # DoubleRow: relT2[k2, e*RELTW + c] = relT[2*(k2 % 32) + e, c] (rows 32..63
    # duplicate 0..31 so odd heads' base partition 32 reads the same table).
    c = np.arange(RELTW)
    rows = RELW - 1 - np.clip(c - (MAX_POS - 1), 0, RELW - 1)
    relT64 = np.ascontiguousarray(rel_emb[rows].T)          # [64, 2047]
    k2 = np.arange(64)[:, None]
    relT2 = np.concatenate([relT64[2 * (k2[:, 0] % 32)],
                            relT64[2 * (k2[:, 0] % 32) + 1]], axis=1)  # [64, 2*2047]
    ident = np.eye(128, dtype=np.float32)
    # id2[k2, e*128 + mcol] = 1 iff (2*k2 + e) == mcol
    id2 = np.zeros((64, 256), dtype=np.float32)
    kk = np.arange(64)
    id2[kk, 2 * kk] = 1.0
    id2[kk, 128 + 2 * kk + 1] = 1.0
    base = {
        "wq": tobf(Wq), "wk": tobf(Wk), "wv": tobf(Wv), "wo": tobf(Wo),
        "relT2": tof8(relT2), "relT8": tof8(np.concatenate([relT64, relT64], 0)),
        "bo": tobf(bo.reshape(1, DIM)),
        "ident": tobf(ident), "id2": tof8(id2),
        "id8": tof8(np.eye(128, dtype=np.float32)),
    }
    in_maps = []
    for c_ in range(B):
        m = dict(base)
        m["xT"] = tobf(np.ascontiguousarray(x[c_].T))
        in_maps.append(m)
    return in_maps


def kernel(x, Wq, Wkv, rel_emb, Wo, bo):
    if "nc" not in _CACHE:
        _CACHE["nc"] = _build_nc()
    nc = _CACHE["nc"]
    in_maps = _prep_inputs(x, Wq, Wkv, rel_emb, Wo, bo)
    res = run_bass_kernel_spmd(nc, in_maps, list(range(B))).results
    out = np.stack([res[c]["out"] for c in range(B)]).astype(np.float32)
    return out
